# revision 33
# baseline (speedup 1.0000x reference)
"""Causal self-attention (RoPE quirk variant) on 8 Trainium2 NeuronCores.

Reference computation (B=2, T=1024, C=2048, H=64 heads, hd=32):
  qkv = x @ w_attn; split q,k,v; RoPE(dim=n_head quirk) on q,k;
  causal softmax attention; y @ w_proj.

Sharding: 8 cores = 2 batches x 4 head-quarter shards (16 heads / core).
Each core computes attention for its 16 heads on its batch and a partial
output projection (its 512 channels of the 2048-channel contraction);
the host sums the 4 partials per batch.

Device design (v3):
  * fp16 everywhere on the 16-bit path; PSUM/z stay fp32.
  * v is stored augmented per head as 64-wide stationary tiles:
    head-even [v(0:32) | ones(32:64)], head-odd [ones(0:32) | v(32:64)].
    A head pair occupies one psum bank as [y0 | z0 | z1 | y1] (pair at
    tile_position cols 0 / 64), so the softmax denominators ride along
    in the PV matmul -- the separate ones-matmul z pass of v2 is gone
    -- while keeping the narrow (128x64) PE tile shape that streams
    ~2x faster than full (128x128) tiles.
  * y normalization without the 3.3us DVE reciprocal: 1/z computed on
    the scalar engine as Exp(-Ln(z)) over the whole psum tile (unused
    rows produce garbage that is never read), then per-head-band
    [32,512] DVE multiplies.
  * attention groups cascade: each segment runs qc1 of group g plus
    qc0 of group g+1 against the next group's projection filler, so
    the ACT-bound exp chains always overlap PE-heavy projections.
  * input DMAs split across sync/scalar/gpsimd queues with the
    group-0 weights split into 4-ko pieces so the first matmuls can
    start as soon as possible.
  * fillers front-loaded (rope emitted before the trailing v half) so
    rope_g completes well before att_g starts.
  * out DMAs on the sync queue (idle at tail); out-proj casts split
    across vector/scalar; seg F borrows the attention psum banks for a
    4-deep out-proj pipeline.
"""

import json
import os
import sys
from itertools import chain

sys.path.insert(0, "/opt/trn_rl_repo")

import numpy as np

import concourse.bass as bass
import concourse.mybir as mybir
import concourse.tile as tile

F32 = mybir.dt.float32
F16 = mybir.dt.float16

INTERLEAVE = True

B, T, C = 2, 1024, 2048
H, HD = 64, 32
SCALE = 1.0 / np.sqrt(32.0)

_PATCHED = False


def _split_excess_waits(bir_json: bytes) -> bytes:
    """The walrus build in this container encodes at most ONE sync-wait per
    instruction; Tile's wait assigner emits several. Hoist excess waits onto
    same-engine NoOps placed immediately before the instruction."""
    d = json.loads(bir_json)
    ctr = 0
    for fn in d.get("functions", []):
        for blk in fn.get("blocks", []):
            out = []
            for inst in blk.get("instructions", []):
                si = inst.get("sync_info")
                waits = (si or {}).get("on_wait") or []
                if len(waits) > 1:
                    for w in waits[:-1]:
                        out.append({
                            "name": f"WSplit-{ctr}",
                            "opcode": "NoOp",
                            "engine": inst["engine"],
                            "ins": [],
                            "outs": [],
                            "sync_info": {"on_update": [], "on_wait": [w]},
                        })
                        ctr += 1
                    si["on_wait"] = [waits[-1]]
                out.append(inst)
            blk["instructions"] = out
    return json.dumps(d).encode()


def _install_patches():
    global _PATCHED
    if _PATCHED:
        return
    import concourse.bass_utils as bu
    import concourse.bass2jax as b2j

    orig = bu.compile_bir_kernel

    def patched_compile(bir_json, tmpdir, neff_name="file.neff"):
        return orig(_split_excess_waits(bir_json), tmpdir, neff_name)

    bu.compile_bir_kernel = patched_compile
    b2j.compile_bir_kernel = patched_compile
    _PATCHED = True


def _build_bass():
    nc = bass.Bass(trn_type="TRN2")
    xT = nc.dram_tensor("xT", [128, 16, 1024], F16, kind="ExternalInput").ap()
    wQK = nc.dram_tensor("wQK", [128, 8, 16, 128], F16, kind="ExternalInput").ap()
    wV = nc.dram_tensor("wV", [128, 16, 512], F16, kind="ExternalInput").ap()
    wP = nc.dram_tensor("wP", [128, 4, 2048], F16, kind="ExternalInput").ap()
    cosT = nc.dram_tensor("cosT", [128, 1024], F16, kind="ExternalInput").ap()
    sinT = nc.dram_tensor("sinT", [128, 1024], F16, kind="ExternalInput").ap()
    tri4 = nc.dram_tensor("tri4", [128, 4, 128], F16, kind="ExternalInput").ap()
    out = nc.dram_tensor("out", [1024, 2048], F16, kind="ExternalOutput").ap()
    outr = out.rearrange("(tq p) n -> tq p n", p=128)
    debug = os.environ.get("KBG_DEBUG") == "1"
    if debug:
        dbgq = nc.dram_tensor("dbgq", [128, 8, 1024], F16,
                              kind="ExternalOutput").ap()
        dbgv = nc.dram_tensor("dbgv", [128, 8, 1024], F16,
                              kind="ExternalOutput").ap()
        dbgy = nc.dram_tensor("dbgy", [128, 4, 1024], F16,
                              kind="ExternalOutput").ap()
        dbge = nc.dram_tensor("dbge", [128, 4, 512], F16,
                              kind="ExternalOutput").ap()

    EXP = mybir.ActivationFunctionType.Exp

    with tile.TileContext(nc) as tc:
        with tc.tile_pool(name="persist", bufs=1) as persist, \
             tc.tile_pool(name="xpool", bufs=1) as xpool, \
             tc.tile_pool(name="wstream", bufs=3) as wsp, \
             tc.tile_pool(name="qtp", bufs=2) as qtp, \
             tc.tile_pool(name="esp", bufs=4) as esp, \
             tc.tile_pool(name="osb", bufs=4) as osb, \
             tc.tile_pool(name="zrp", bufs=2) as zrp, \
             tc.tile_pool(name="psA", bufs=1, space="PSUM") as psa, \
             tc.tile_pool(name="psS", bufs=2, space="PSUM") as psS, \
             tc.tile_pool(name="psY", bufs=1, space="PSUM") as psY:

            qkT = persist.tile([128, 8, 1024], F16)     # rotated q (0-3) / k (4-7)
            # v augmented: per (kb, head) a 64-wide stationary tile
            # (see module docstring).
            v_sb = persist.tile([128, 8, 16, 64], F16)
            cos_sb = persist.tile([128, 1024], F16)
            sin_sb = persist.tile([128, 1024], F16)     # sign-folded
            tri_sb = persist.tile([128, 4, 128], F16)
            wv_sb = persist.tile([128, 16, 512], F16)
            wp_sb = persist.tile([128, 4, 2048], F16)
            warm = persist.tile([128, 16], F32)
            y_tiles = [persist.tile([128, 1024], F16, name=f"y{g}")
                       for g in range(4)]
            xt = xpool.tile([128, 16, 1024], F16)

            # --- bulk input DMAs first (descriptor pipelines take ~6-9us
            # to produce data; issue the critical path ASAP).
            # sync queue: group-0 q weights (4 small pieces), then even x
            # slices; scalar queue: odd x slices, group-0 k weights, the
            # rope/mask tables, then v weights.  Splitting x across both
            # queues halves the issue-serialization on the critical path.
            wt_first = [wsp.tile([128, 16, 128], F16, tag="wa", name=f"wt0_{j}")
                        for j in range(2)]
            for p in range(4):
                nc.sync.dma_start(wt_first[0][:, 4 * p:4 * p + 4, :],
                                  wQK[:, 0, 4 * p:4 * p + 4, :])
            for ko in range(0, 16, 2):
                nc.sync.dma_start(xt[:, ko, :], xT[:, ko, :])
            for ko in range(1, 16, 2):
                nc.scalar.dma_start(xt[:, ko, :], xT[:, ko, :])
            for p in range(4):
                nc.scalar.dma_start(wt_first[1][:, 4 * p:4 * p + 4, :],
                                    wQK[:, 4, 4 * p:4 * p + 4, :])
            nc.scalar.dma_start(cos_sb, cosT)
            nc.scalar.dma_start(sin_sb, sinT)
            nc.scalar.dma_start(tri_sb, tri4)
            for p in range(2):
                nc.scalar.dma_start(wv_sb[:, 8 * p:8 * p + 8, :],
                                    wV[:, 8 * p:8 * p + 8, :])

            # --- prelude compute: ones bands of v_aug + ACT exp warm-up.
            # Each head's 64-wide stationary tile is [v | 1] (even) /
            # [1 | v] (odd); the v copies fill the v bands later.
            ones_view = v_sb.rearrange("p a (pr tw) c -> p (a pr) tw c", tw=2)
            nc.vector.memset(ones_view[:, :, 0, 32:64], 1.0)
            nc.vector.memset(ones_view[:, :, 1, 0:32], 1.0)
            nc.vector.memset(warm, 0.0)
            nc.scalar.activation(warm, warm, EXP)

            # ---------- generators (each yield ~ one PE-instruction step) ----

            def gen_qk_dma(g):
                if g == 0:
                    yield
                    return
                for j, mi in enumerate((g, 4 + g)):
                    wt = wsp.tile([128, 16, 128], F16, tag="wa",
                                  name=f"wt{g}_{j}")
                    nc.sync.dma_start(wt, wQK[:, mi])
                    gen_qk_dma.cache[(g, j)] = wt
                    yield
            gen_qk_dma.cache = {}

            def gen_qkrope(g):
                """q/k projection for group g, with each j's RoPE emitted
                right after that j's projection so the rope chain of j=0
                overlaps the j=1 matmuls."""
                pre = qtp.tile([128, 2, 1024], F16, tag="pre")
                swp = qtp.tile([128, 2, 1024], F16, tag="swp")
                pre_v = pre.rearrange("(a b) j f -> a b j f", b=2)
                swp_v = swp.rearrange("(a b) j f -> a b j f", b=2)
                for j, dst in enumerate((qkT[:, g, :], qkT[:, 4 + g, :])):
                    if g == 0:
                        wt = wt_first[j]
                    else:
                        wt = gen_qk_dma.cache[(g, j)]
                    for half in range(2):
                        ps = psa.tile([128, 512], F32, tag=f"psA{half}",
                                      name=f"qk{g}_{j}_{half}")
                        c0 = half * 512
                        for ko in range(16):
                            nc.tensor.matmul(ps, wt[:, ko, :],
                                             xt[:, ko, c0:c0 + 512],
                                             start=ko == 0, stop=ko == 15)
                            yield
                        nc.vector.tensor_copy(pre[:, j, c0:c0 + 512], ps)
                        yield
                    nc.sync.dma_start(swp_v[:, 0, j], pre_v[:, 1, j])
                    nc.sync.dma_start(swp_v[:, 1, j], pre_v[:, 0, j])
                    yield
                    nc.gpsimd.tensor_mul(swp[:, j, :], swp[:, j, :], sin_sb)
                    yield
                    nc.vector.tensor_mul(pre[:, j, :], pre[:, j, :], cos_sb)
                    yield
                    nc.vector.tensor_add(dst, pre[:, j, :], swp[:, j, :])
                    yield

            def gen_v(tb_lo, tb_hi):
                """x-stationary projection: v in natural [T, chan] layout,
                scattered into the augmented v_sb (head stride 64)."""
                for tb in range(tb_lo, tb_hi):
                    psv = psa.tile([128, 512], F32, tag=f"psA{tb % 2}",
                                   name=f"v{tb}")
                    for ko in range(16):
                        nc.tensor.matmul(psv,
                                         xt[:, ko, tb * 128:(tb + 1) * 128],
                                         wv_sb[:, ko, :],
                                         start=ko == 0, stop=ko == 15)
                        yield
                    # scatter: head-even v -> cols 0:32, head-odd -> 32:64
                    src = psv.rearrange("p (pr tw c) -> p pr tw c", tw=2, c=32)
                    dst = v_sb.rearrange(
                        "p a (pr tw) c -> p a pr tw c", tw=2)[:, tb]
                    if tb < 4:
                        nc.scalar.copy(dst[:, :, 0, 0:32], src[:, :, 0])
                        nc.scalar.copy(dst[:, :, 1, 32:64], src[:, :, 1])
                    else:
                        nc.vector.tensor_copy(dst[:, :, 0, 0:32], src[:, :, 0])
                        nc.vector.tensor_copy(dst[:, :, 1, 32:64], src[:, :, 1])
                    yield

            def gen_wp_dma():
                nc.scalar.dma_start(wp_sb, wP)
                yield

            def emit_pv(g, kb, es, N, off, psyA, psyB, start, stop):
                # head pair (2h, 2h+1) in one bank: even at cols 0 (rows
                # [y0|z0]), odd at cols 64 (rows [z1|y1]).
                for h in range(4):
                    psy = psyA if h < 2 else psyB
                    pos = 64 * (h % 2)
                    nc.tensor.matmul(psy[pos:pos + 64, off:512],
                                     v_sb[:, kb, 4 * g + h, :],
                                     es[:, h, :N],
                                     start=start, stop=stop,
                                     tile_position=(0, pos),
                                     skip_group_check=True)

            def gen_att(g, qcs):
                """Attention blocks for group g.  Scores/exp run at head-pair
                granularity: each pair owns a full psum bank per head, and the
                2-bank pair tiles are double-buffered so the next block's
                score matmuls overlap the current block's ACT exp."""
                y_g = y_tiles[g]
                for qc in qcs:
                    q0 = qc * 512
                    nkb = (qc + 1) * 4
                    psyA = psY.tile([128, 512], F32, tag="psyA",
                                    name=f"psyA{g}_{qc}")
                    psyB = psY.tile([128, 512], F32, tag="psyB",
                                    name=f"psyB{g}_{qc}")
                    prev = None
                    for kb in range(nkb):
                        k0 = kb * 128
                        n0 = max(q0, k0)
                        N = q0 + 512 - n0
                        off = n0 - q0
                        es = esp.tile([128, 4, 512], F16, tag="es")
                        pss2 = []
                        for hp in range(2):
                            pss = psS.tile([128, 2, 512], F32, tag="pss")
                            pss2.append(pss)
                            for j in range(2):
                                h = 2 * hp + j
                                nc.tensor.matmul(
                                    pss[:, j, :N],
                                    qkT[32 * h:32 * h + 32, 4 + g,
                                        k0:k0 + 128],
                                    qkT[32 * h:32 * h + 32, g, n0:n0 + N],
                                    start=True, stop=True,
                                    tile_position=(32 * h, 0))
                        for hp in range(2):
                            nc.scalar.activation(
                                es[:, 2 * hp:2 * hp + 2, :N],
                                pss2[hp][:, :, :N], EXP)
                        if prev is not None:
                            emit_pv(*prev)
                        if k0 >= q0:
                            nc.vector.tensor_mul(es[:, :, 0:128],
                                                 es[:, :, 0:128], tri_sb)
                        if debug and g == 0 and qc == 0 and kb == 0:
                            nc.sync.dma_start(dbge, es)
                        prev = (g, kb, es, N, off, psyA, psyB, kb == 0,
                                kb == nkb - 1)
                        yield
                    emit_pv(*prev)
                    # y normalization straight from psum: 1/z = Exp(-Ln z)
                    # on the scalar engine over the whole tile (y rows give
                    # garbage that is never read), then per-head band muls.
                    for pi, psy in enumerate((psyA, psyB)):
                        lnz = zrp.tile([128, 512], F32, tag=f"ln{pi}",
                                       bufs=1, name=f"ln{pi}_{g}_{qc}")
                        zr = zrp.tile([128, 512], F32, tag=f"zr{pi}",
                                      name=f"zr{pi}_{g}_{qc}")
                        nc.scalar.activation(lnz, psy,
                                             mybir.ActivationFunctionType.Ln)
                        nc.scalar.activation(zr, lnz, EXP, scale=-1.0)
                        nc.vector.tensor_mul(
                            y_g[64 * pi:64 * pi + 32, q0:q0 + 512],
                            psy[0:32, :], zr[32:64, :])
                        nc.vector.tensor_mul(
                            y_g[64 * pi + 32:64 * pi + 64, q0:q0 + 512],
                            psy[96:128, :], zr[64:96, :])
                    yield

            def gen_outproj(tq_list, seg_f=False):
                pso_tags = [(psa, "psA0"), (psa, "psA1")]
                if seg_f:
                    pso_tags += [(psY, "psyA"), (psY, "psyB")]
                ctr = 0
                for tq in tq_list:
                    for p in range(2):
                        pso = []
                        for n in range(2):
                            pool, tg = pso_tags[ctr % len(pso_tags)]
                            ctr += 1
                            pso.append(pool.tile([128, 512], F32, tag=tg,
                                                 name=f"pso{tq}_{p}_{n}"))
                        for gk in range(4):
                            lhs = y_tiles[gk][:, tq * 128:(tq + 1) * 128]
                            for n in range(2):
                                nc.tensor.matmul(
                                    pso[n], lhs,
                                    wp_sb[:, gk, (2 * p + n) * 512:
                                          (2 * p + n + 1) * 512],
                                    start=gk == 0, stop=gk == 3)
                                yield
                        for n in range(2):
                            o_sb = osb.tile([128, 512], F16, tag="osb")
                            if n == 1:
                                nc.scalar.copy(o_sb, pso[n])
                            else:
                                nc.vector.tensor_copy(o_sb, pso[n])
                            nc.scalar.dma_start(
                                outr[tq][:, (2 * p + n) * 512:
                                         (2 * p + n + 1) * 512], o_sb)
                            yield

            # ---------- driver: interleave attention with filler PE work ----

            def run(gen):
                for _ in gen:
                    pass

            def co_run(att_gen, filler, per_block):
                """One attention block, then `per_block` filler steps."""
                if not INTERLEAVE:
                    run(filler)
                    run(att_gen)
                    return
                for _ in att_gen:
                    for _ in range(per_block):
                        if next(filler, StopIteration) is StopIteration:
                            break
                run(filler)

            # seg A: q/k proj of group 0 + RoPE0 + first half of v
            run(gen_qkrope(0))
            run(gen_v(0, 4))

            # Interleaved segments.  EMISSION-ORDER INVARIANT: a consumer
            # must be emitted after its producer (Tile only creates deps in
            # program order) -- so v(4,8) is emitted before att0.qc1's PV
            # blocks, and each rope_g before att_g's first scores.  From
            # seg C on, segments cascade: qc1 of group g plus qc0 of group
            # g+1 run against the next group's projection filler, so the
            # ACT-bound exp chains always overlap PE-heavy projections.
            filler_b = chain(gen_qk_dma(1), gen_v(4, 8), gen_qkrope(1))
            co_run(gen_att(0, (0, 1)), filler_b, 11)

            filler_c = chain(gen_qk_dma(2), gen_qkrope(2), gen_wp_dma())
            co_run(chain(gen_att(1, (0, 1)), gen_att(2, (0,))), filler_c, 6)

            filler_d = chain(gen_qk_dma(3), gen_qkrope(3))
            co_run(chain(gen_att(2, (1,)), gen_att(3, (0,))), filler_d, 9)

            # seg E: att3 qc1 interleaved with the first half of the
            # out-projection (tq 0-3 only need y3 columns from qc0).
            co_run(gen_att(3, (1,)), gen_outproj((0, 1, 2, 3)), 9)

            # seg F: remaining out-projection (borrows attention psum banks
            # for a 4-deep pipeline; casts on scalar -- ACT is idle now).
            run(gen_outproj((4, 5, 6, 7), seg_f=True))

            if debug:
                nc.sync.dma_start(dbgq, qkT)
                nc.sync.dma_start(dbgv,
                                  v_sb.rearrange("p a h c -> p a (h c)"))
                for g in range(4):
                    nc.sync.dma_start(dbgy[:, g], y_tiles[g])
    return nc


_NC_CACHE = None


def _host_inputs(x, pos, w_attn, w_proj):
    """Build the 8 per-core input dicts."""
    x = np.asarray(x, dtype=np.float32)
    pos = np.asarray(pos, dtype=np.float32)
    w_attn = np.asarray(w_attn, dtype=np.float32)
    w_proj = np.asarray(w_proj, dtype=np.float32)

    TRI = (np.arange(128)[:, None] <= np.arange(128)[None, :]).astype(
        np.float16)
    tri4 = np.ascontiguousarray(np.tile(TRI[:, None, :], (1, 4, 1)))
    inv_freq = (1.0 / (10000.0 ** (np.arange(0, H, 2, dtype=np.float32) / H)))
    sinus = pos[:, None] * inv_freq[None, :]              # [T, 32]
    cosT = np.tile(np.cos(sinus).T, (4, 1))               # [128, T]
    sinT = np.tile(np.sin(sinus).T, (4, 1)).copy()
    sinT[0::2, :] *= -1.0                                 # rotate_half signs
    cosT = cosT.astype(np.float16)
    sinT = sinT.astype(np.float16)

    in_maps = []
    for core in range(8):
        b, gq = divmod(core, 4)
        hs = slice(gq * 512, (gq + 1) * 512)
        Wq = (w_attn[:, 0:2048][:, hs] * SCALE).astype(np.float32)
        Wk = w_attn[:, 2048:4096][:, hs]
        Wv = w_attn[:, 4096:6144][:, hs]
        WQK = np.concatenate([Wq, Wk], axis=1)            # [2048, 1024]
        wqk = np.ascontiguousarray(
            WQK.reshape(16, 128, 8, 128).transpose(1, 2, 0, 3)).astype(
            np.float16)                                   # ki mi ko mc
        wv = np.ascontiguousarray(
            Wv.reshape(16, 128, 512).transpose(1, 0, 2)).astype(np.float16)
        wPr = np.ascontiguousarray(
            w_proj[hs, :].reshape(4, 128, 2048).transpose(1, 0, 2)).astype(
            np.float16)
        xTr = np.ascontiguousarray(
            x[b].T.reshape(16, 128, 1024).transpose(1, 0, 2)).astype(
            np.float16)
        in_maps.append({
            "xT": xTr, "wQK": wqk, "wV": wv, "wP": wPr,
            "cosT": cosT, "sinT": sinT, "tri4": tri4,
        })
    return in_maps


def kernel(x, pos, w_attn, w_proj, _trace=False):
    global _NC_CACHE
    _install_patches()
    from concourse.bass_utils import run_bass_kernel_spmd

    if _NC_CACHE is None:
        _NC_CACHE = _build_bass()
    nc = _NC_CACHE
    in_maps = _host_inputs(x, pos, w_attn, w_proj)
    res = run_bass_kernel_spmd(nc, in_maps, core_ids=list(range(8)), trace=_trace)
    outs = [np.asarray(res.results[c]["out"], dtype=np.float32)
            for c in range(8)]
    full = np.stack([
        outs[0] + outs[1] + outs[2] + outs[3],
        outs[4] + outs[5] + outs[6] + outs[7],
    ]).astype(np.float32)
    kernel.last_results = res
    return full


# revision 34
# speedup vs baseline: 1.0479x; 1.0479x over previous
"""Causal self-attention (RoPE quirk variant) on 8 Trainium2 NeuronCores.

Reference computation (B=2, T=1024, C=2048, H=64 heads, hd=32):
  qkv = x @ w_attn; split q,k,v; RoPE(dim=n_head quirk) on q,k;
  causal softmax attention; y @ w_proj.

Sharding: 8 cores = 2 batches x 4 head-quarter shards (16 heads / core).
Each core computes attention for its 16 heads on its batch and a partial
output projection (its 512 channels of the 2048-channel contraction);
the host sums the 4 partials per batch.

Device design (v3):
  * fp16 everywhere on the 16-bit path; PSUM/z stay fp32.
  * v is stored augmented per head as 64-wide stationary tiles:
    head-even [v(0:32) | ones(32:64)], head-odd [ones(0:32) | v(32:64)].
    A head pair occupies one psum bank as [y0 | z0 | z1 | y1] (pair at
    tile_position cols 0 / 64), so the softmax denominators ride along
    in the PV matmul -- the separate ones-matmul z pass of v2 is gone
    -- while keeping the narrow (128x64) PE tile shape that streams
    ~2x faster than full (128x128) tiles.
  * y normalization without the 3.3us DVE reciprocal: 1/z computed on
    the scalar engine as Exp(-Ln(z)) over the whole psum tile (unused
    rows produce garbage that is never read), then per-head-band
    [32,512] DVE multiplies.
  * attention groups cascade: each segment runs qc1 of group g plus
    qc0 of group g+1 against the next group's projection filler, so
    the ACT-bound exp chains always overlap PE-heavy projections.
  * input DMAs split across sync/scalar/gpsimd queues with the
    group-0 weights split into 4-ko pieces so the first matmuls can
    start as soon as possible.
  * fillers front-loaded (rope emitted before the trailing v half) so
    rope_g completes well before att_g starts.
  * out DMAs on the sync queue (idle at tail); out-proj casts split
    across vector/scalar; seg F borrows the attention psum banks for a
    4-deep out-proj pipeline.
"""

import json
import os
import sys
from itertools import chain

sys.path.insert(0, "/opt/trn_rl_repo")

import numpy as np

import concourse.bass as bass
import concourse.mybir as mybir
import concourse.tile as tile

F32 = mybir.dt.float32
F16 = mybir.dt.float16

INTERLEAVE = True

B, T, C = 2, 1024, 2048
H, HD = 64, 32
SCALE = 1.0 / np.sqrt(32.0)

_PATCHED = False


def _split_excess_waits(bir_json: bytes) -> bytes:
    """The walrus build in this container encodes at most ONE sync-wait per
    instruction; Tile's wait assigner emits several. Hoist excess waits onto
    same-engine NoOps placed immediately before the instruction."""
    d = json.loads(bir_json)
    ctr = 0
    for fn in d.get("functions", []):
        for blk in fn.get("blocks", []):
            out = []
            for inst in blk.get("instructions", []):
                si = inst.get("sync_info")
                waits = (si or {}).get("on_wait") or []
                if len(waits) > 1:
                    for w in waits[:-1]:
                        out.append({
                            "name": f"WSplit-{ctr}",
                            "opcode": "NoOp",
                            "engine": inst["engine"],
                            "ins": [],
                            "outs": [],
                            "sync_info": {"on_update": [], "on_wait": [w]},
                        })
                        ctr += 1
                    si["on_wait"] = [waits[-1]]
                out.append(inst)
            blk["instructions"] = out
    return json.dumps(d).encode()


def _install_patches():
    global _PATCHED
    if _PATCHED:
        return
    import concourse.bass_utils as bu
    import concourse.bass2jax as b2j

    orig = bu.compile_bir_kernel

    def patched_compile(bir_json, tmpdir, neff_name="file.neff"):
        return orig(_split_excess_waits(bir_json), tmpdir, neff_name)

    bu.compile_bir_kernel = patched_compile
    b2j.compile_bir_kernel = patched_compile
    _PATCHED = True


def _build_bass():
    nc = bass.Bass(trn_type="TRN2")
    xT = nc.dram_tensor("xT", [128, 16, 1024], F16, kind="ExternalInput").ap()
    wQK = nc.dram_tensor("wQK", [128, 8, 16, 128], F16, kind="ExternalInput").ap()
    wV = nc.dram_tensor("wV", [128, 16, 512], F16, kind="ExternalInput").ap()
    wP = nc.dram_tensor("wP", [128, 4, 2048], F16, kind="ExternalInput").ap()
    cosT = nc.dram_tensor("cosT", [128, 1024], F16, kind="ExternalInput").ap()
    sinT = nc.dram_tensor("sinT", [128, 1024], F16, kind="ExternalInput").ap()
    tri4 = nc.dram_tensor("tri4", [128, 4, 128], F16, kind="ExternalInput").ap()
    out = nc.dram_tensor("out", [1024, 2048], F16, kind="ExternalOutput").ap()
    outr = out.rearrange("(tq p) n -> tq p n", p=128)
    debug = os.environ.get("KBG_DEBUG") == "1"
    if debug:
        dbgq = nc.dram_tensor("dbgq", [128, 8, 1024], F16,
                              kind="ExternalOutput").ap()
        dbgv = nc.dram_tensor("dbgv", [128, 8, 1024], F16,
                              kind="ExternalOutput").ap()
        dbgy = nc.dram_tensor("dbgy", [128, 4, 1024], F16,
                              kind="ExternalOutput").ap()
        dbge = nc.dram_tensor("dbge", [128, 4, 512], F16,
                              kind="ExternalOutput").ap()

    EXP = mybir.ActivationFunctionType.Exp

    with tile.TileContext(nc) as tc:
        with tc.tile_pool(name="persist", bufs=1) as persist, \
             tc.tile_pool(name="xpool", bufs=1) as xpool, \
             tc.tile_pool(name="wstream", bufs=3) as wsp, \
             tc.tile_pool(name="qtp", bufs=2) as qtp, \
             tc.tile_pool(name="esp", bufs=4) as esp, \
             tc.tile_pool(name="osb", bufs=4) as osb, \
             tc.tile_pool(name="zrp", bufs=2) as zrp, \
             tc.tile_pool(name="psA", bufs=1, space="PSUM") as psa, \
             tc.tile_pool(name="psS", bufs=2, space="PSUM") as psS, \
             tc.tile_pool(name="psY", bufs=1, space="PSUM") as psY:

            qkT = persist.tile([128, 8, 1024], F16)     # rotated q (0-3) / k (4-7)
            # v augmented: per (kb, head) a 64-wide stationary tile
            # (see module docstring).
            v_sb = persist.tile([128, 8, 16, 64], F16)
            cos_sb = persist.tile([128, 1024], F16)
            sin_sb = persist.tile([128, 1024], F16)     # sign-folded
            tri_sb = persist.tile([128, 4, 128], F16)
            wv_sb = persist.tile([128, 16, 512], F16)
            wp_sb = persist.tile([128, 4, 2048], F16)
            warm = persist.tile([128, 16], F32)
            y_tiles = [persist.tile([128, 1024], F16, name=f"y{g}")
                       for g in range(4)]
            xt = xpool.tile([128, 16, 1024], F16)

            # --- bulk input DMAs first (descriptor pipelines take ~6-9us
            # to produce data; issue the critical path ASAP).
            # sync queue: group-0 q weights (4 small pieces), then even x
            # slices; scalar queue: odd x slices, group-0 k weights, the
            # rope/mask tables, then v weights.  Splitting x across both
            # queues halves the issue-serialization on the critical path.
            wt_first = [wsp.tile([128, 16, 128], F16, tag="wa", name=f"wt0_{j}")
                        for j in range(2)]
            for p in range(4):
                nc.sync.dma_start(wt_first[0][:, 4 * p:4 * p + 4, :],
                                  wQK[:, 0, 4 * p:4 * p + 4, :])
            for ko in range(0, 16, 2):
                nc.sync.dma_start(xt[:, ko, :], xT[:, ko, :])
            for ko in range(1, 16, 2):
                nc.scalar.dma_start(xt[:, ko, :], xT[:, ko, :])
            for p in range(4):
                nc.scalar.dma_start(wt_first[1][:, 4 * p:4 * p + 4, :],
                                    wQK[:, 4, 4 * p:4 * p + 4, :])
            nc.scalar.dma_start(cos_sb, cosT)
            nc.scalar.dma_start(sin_sb, sinT)
            nc.scalar.dma_start(tri_sb, tri4)
            for p in range(2):
                nc.scalar.dma_start(wv_sb[:, 8 * p:8 * p + 8, :],
                                    wV[:, 8 * p:8 * p + 8, :])

            # --- prelude compute: ones bands of v_aug + ACT exp warm-up.
            # Each head's 64-wide stationary tile is [v | 1] (even) /
            # [1 | v] (odd); the v copies fill the v bands later.
            ones_view = v_sb.rearrange("p a (pr tw) c -> p (a pr) tw c", tw=2)
            nc.vector.memset(ones_view[:, :, 0, 32:64], 1.0)
            nc.vector.memset(ones_view[:, :, 1, 0:32], 1.0)
            nc.vector.memset(warm, 0.0)
            nc.scalar.activation(warm, warm, EXP)

            # ---------- generators (each yield ~ one PE-instruction step) ----

            def gen_qk_dma(g):
                if g == 0:
                    yield
                    return
                for j, mi in enumerate((g, 4 + g)):
                    wt = wsp.tile([128, 16, 128], F16, tag="wa",
                                  name=f"wt{g}_{j}")
                    nc.sync.dma_start(wt, wQK[:, mi])
                    gen_qk_dma.cache[(g, j)] = wt
                    yield
            gen_qk_dma.cache = {}

            def gen_qk_mms(g):
                """q/k projection for group g into pre, then RoPE -> qkT."""
                pre = qtp.tile([128, 2, 1024], F16, tag="pre")
                swp = qtp.tile([128, 2, 1024], F16, tag="swp")
                for j in range(2):
                    if g == 0:
                        wt = wt_first[j]
                    else:
                        wt = gen_qk_dma.cache[(g, j)]
                    for half in range(2):
                        ps = psa.tile([128, 512], F32, tag=f"psA{half}",
                                      name=f"qk{g}_{j}_{half}")
                        c0 = half * 512
                        for ko in range(16):
                            nc.tensor.matmul(ps, wt[:, ko, :],
                                             xt[:, ko, c0:c0 + 512],
                                             start=ko == 0, stop=ko == 15)
                            yield
                        nc.vector.tensor_copy(pre[:, j, c0:c0 + 512], ps)
                        yield
                gen_qk_mms.pre[g] = (pre, swp)
            gen_qk_mms.pre = {}

            def gen_rope(g):
                pre, swp = gen_qk_mms.pre[g]
                pre_v = pre.rearrange("(a b) j f -> a b j f", b=2)
                swp_v = swp.rearrange("(a b) j f -> a b j f", b=2)
                nc.sync.dma_start(swp_v[:, 0], pre_v[:, 1])
                nc.sync.dma_start(swp_v[:, 1], pre_v[:, 0])
                yield
                for j, dst in enumerate((qkT[:, g, :], qkT[:, 4 + g, :])):
                    nc.gpsimd.tensor_mul(swp[:, j, :], swp[:, j, :], sin_sb)
                    yield
                    nc.vector.tensor_mul(pre[:, j, :], pre[:, j, :], cos_sb)
                    yield
                    nc.vector.tensor_add(dst, pre[:, j, :], swp[:, j, :])
                    yield

            def gen_v(tb_lo, tb_hi):
                """x-stationary projection: v in natural [T, chan] layout,
                scattered into the augmented v_sb (head stride 64)."""
                for tb in range(tb_lo, tb_hi):
                    psv = psa.tile([128, 512], F32, tag=f"psA{tb % 2}",
                                   name=f"v{tb}")
                    for ko in range(16):
                        nc.tensor.matmul(psv,
                                         xt[:, ko, tb * 128:(tb + 1) * 128],
                                         wv_sb[:, ko, :],
                                         start=ko == 0, stop=ko == 15)
                        yield
                    # scatter: head-even v -> cols 0:32, head-odd -> 32:64
                    src = psv.rearrange("p (pr tw c) -> p pr tw c", tw=2, c=32)
                    dst = v_sb.rearrange(
                        "p a (pr tw) c -> p a pr tw c", tw=2)[:, tb]
                    if tb < 4:
                        nc.scalar.copy(dst[:, :, 0, 0:32], src[:, :, 0])
                        nc.scalar.copy(dst[:, :, 1, 32:64], src[:, :, 1])
                    else:
                        nc.vector.tensor_copy(dst[:, :, 0, 0:32], src[:, :, 0])
                        nc.vector.tensor_copy(dst[:, :, 1, 32:64], src[:, :, 1])
                    yield

            def gen_wp_dma():
                nc.scalar.dma_start(wp_sb, wP)
                yield

            def emit_pv(g, kb, es, N, off, psyA, psyB, start, stop):
                # head pair (2h, 2h+1) in one bank: even at cols 0 (rows
                # [y0|z0]), odd at cols 64 (rows [z1|y1]).
                for h in range(4):
                    psy = psyA if h < 2 else psyB
                    pos = 64 * (h % 2)
                    nc.tensor.matmul(psy[pos:pos + 64, off:512],
                                     v_sb[:, kb, 4 * g + h, :],
                                     es[:, h, :N],
                                     start=start, stop=stop,
                                     tile_position=(0, pos),
                                     skip_group_check=True)

            def gen_att(g, qcs):
                """Attention blocks for group g.  Scores/exp run at head-pair
                granularity: each pair owns a full psum bank per head, and the
                2-bank pair tiles are double-buffered so the next block's
                score matmuls overlap the current block's ACT exp."""
                y_g = y_tiles[g]
                for qc in qcs:
                    q0 = qc * 512
                    nkb = (qc + 1) * 4
                    psyA = psY.tile([128, 512], F32, tag="psyA",
                                    name=f"psyA{g}_{qc}")
                    psyB = psY.tile([128, 512], F32, tag="psyB",
                                    name=f"psyB{g}_{qc}")
                    prev = None
                    for kb in range(nkb):
                        k0 = kb * 128
                        n0 = max(q0, k0)
                        N = q0 + 512 - n0
                        off = n0 - q0
                        es = esp.tile([128, 4, 512], F16, tag="es")
                        pss2 = []
                        for hp in range(2):
                            pss = psS.tile([128, 2, 512], F32, tag="pss")
                            pss2.append(pss)
                            for j in range(2):
                                h = 2 * hp + j
                                nc.tensor.matmul(
                                    pss[:, j, :N],
                                    qkT[32 * h:32 * h + 32, 4 + g,
                                        k0:k0 + 128],
                                    qkT[32 * h:32 * h + 32, g, n0:n0 + N],
                                    start=True, stop=True,
                                    tile_position=(32 * h, 0))
                        for hp in range(2):
                            nc.scalar.activation(
                                es[:, 2 * hp:2 * hp + 2, :N],
                                pss2[hp][:, :, :N], EXP)
                        if prev is not None:
                            emit_pv(*prev)
                        if k0 >= q0:
                            nc.vector.tensor_mul(es[:, :, 0:128],
                                                 es[:, :, 0:128], tri_sb)
                        if debug and g == 0 and qc == 0 and kb == 0:
                            nc.sync.dma_start(dbge, es)
                        prev = (g, kb, es, N, off, psyA, psyB, kb == 0,
                                kb == nkb - 1)
                        yield
                    emit_pv(*prev)
                    # y normalization straight from psum: 1/z = Exp(-Ln z)
                    # on the scalar engine over the whole tile (y rows give
                    # garbage that is never read), then per-head band muls.
                    for pi, psy in enumerate((psyA, psyB)):
                        lnz = zrp.tile([128, 512], F32, tag=f"ln{pi}",
                                       bufs=1, name=f"ln{pi}_{g}_{qc}")
                        zr = zrp.tile([128, 512], F32, tag=f"zr{pi}",
                                      name=f"zr{pi}_{g}_{qc}")
                        nc.scalar.activation(lnz, psy,
                                             mybir.ActivationFunctionType.Ln)
                        nc.scalar.activation(zr, lnz, EXP, scale=-1.0)
                        nc.vector.tensor_mul(
                            y_g[64 * pi:64 * pi + 32, q0:q0 + 512],
                            psy[0:32, :], zr[32:64, :])
                        nc.vector.tensor_mul(
                            y_g[64 * pi + 32:64 * pi + 64, q0:q0 + 512],
                            psy[96:128, :], zr[64:96, :])
                    yield

            def gen_outproj(tq_list, seg_f=False):
                pso_tags = [(psa, "psA0"), (psa, "psA1")]
                if seg_f:
                    pso_tags += [(psY, "psyA"), (psY, "psyB")]
                ctr = 0
                for tq in tq_list:
                    for p in range(2):
                        pso = []
                        for n in range(2):
                            pool, tg = pso_tags[ctr % len(pso_tags)]
                            ctr += 1
                            pso.append(pool.tile([128, 512], F32, tag=tg,
                                                 name=f"pso{tq}_{p}_{n}"))
                        for gk in range(4):
                            lhs = y_tiles[gk][:, tq * 128:(tq + 1) * 128]
                            for n in range(2):
                                nc.tensor.matmul(
                                    pso[n], lhs,
                                    wp_sb[:, gk, (2 * p + n) * 512:
                                          (2 * p + n + 1) * 512],
                                    start=gk == 0, stop=gk == 3)
                                yield
                        for n in range(2):
                            o_sb = osb.tile([128, 512], F16, tag="osb")
                            if n == 1:
                                nc.scalar.copy(o_sb, pso[n])
                            else:
                                nc.vector.tensor_copy(o_sb, pso[n])
                            nc.scalar.dma_start(
                                outr[tq][:, (2 * p + n) * 512:
                                         (2 * p + n + 1) * 512], o_sb)
                            yield

            # ---------- driver: interleave attention with filler PE work ----

            def run(gen):
                for _ in gen:
                    pass

            def co_run(att_gen, filler, per_block):
                """One attention block, then `per_block` filler steps."""
                if not INTERLEAVE:
                    run(filler)
                    run(att_gen)
                    return
                for _ in att_gen:
                    for _ in range(per_block):
                        if next(filler, StopIteration) is StopIteration:
                            break
                run(filler)

            # seg A: q/k proj of group 0 + RoPE0 + first half of v
            run(gen_qk_mms(0))
            run(gen_rope(0))
            run(gen_v(0, 4))

            # Interleaved segments.  EMISSION-ORDER INVARIANT: a consumer
            # must be emitted after its producer (Tile only creates deps in
            # program order) -- so v(4,8) is emitted before att0.qc1's PV
            # blocks, and each rope_g before att_g's first scores.  From
            # seg C on, segments cascade: qc1 of group g plus qc0 of group
            # g+1 run against the next group's projection filler, so the
            # ACT-bound exp chains always overlap PE-heavy projections.
            filler_b = chain(gen_qk_dma(1), gen_v(4, 8), gen_qk_mms(1),
                             gen_rope(1))
            co_run(gen_att(0, (0, 1)), filler_b, 11)

            filler_c = chain(gen_qk_dma(2), gen_qk_mms(2), gen_rope(2),
                             gen_wp_dma())
            co_run(chain(gen_att(1, (0, 1)), gen_att(2, (0,))), filler_c, 6)

            filler_d = chain(gen_qk_dma(3), gen_qk_mms(3), gen_rope(3))
            co_run(chain(gen_att(2, (1,)), gen_att(3, (0,))), filler_d, 9)

            # seg E: att3 qc1 interleaved with the first half of the
            # out-projection (tq 0-3 only need y3 columns from qc0).
            co_run(gen_att(3, (1,)), gen_outproj((0, 1, 2, 3)), 9)

            # seg F: remaining out-projection (borrows attention psum banks
            # for a 4-deep pipeline; casts on scalar -- ACT is idle now).
            run(gen_outproj((4, 5, 6, 7), seg_f=True))

            if debug:
                nc.sync.dma_start(dbgq, qkT)
                nc.sync.dma_start(dbgv,
                                  v_sb.rearrange("p a h c -> p a (h c)"))
                for g in range(4):
                    nc.sync.dma_start(dbgy[:, g], y_tiles[g])
    return nc


_NC_CACHE = None


def _host_inputs(x, pos, w_attn, w_proj):
    """Build the 8 per-core input dicts."""
    x = np.asarray(x, dtype=np.float32)
    pos = np.asarray(pos, dtype=np.float32)
    w_attn = np.asarray(w_attn, dtype=np.float32)
    w_proj = np.asarray(w_proj, dtype=np.float32)

    TRI = (np.arange(128)[:, None] <= np.arange(128)[None, :]).astype(
        np.float16)
    tri4 = np.ascontiguousarray(np.tile(TRI[:, None, :], (1, 4, 1)))
    inv_freq = (1.0 / (10000.0 ** (np.arange(0, H, 2, dtype=np.float32) / H)))
    sinus = pos[:, None] * inv_freq[None, :]              # [T, 32]
    cosT = np.tile(np.cos(sinus).T, (4, 1))               # [128, T]
    sinT = np.tile(np.sin(sinus).T, (4, 1)).copy()
    sinT[0::2, :] *= -1.0                                 # rotate_half signs
    cosT = cosT.astype(np.float16)
    sinT = sinT.astype(np.float16)

    in_maps = []
    for core in range(8):
        b, gq = divmod(core, 4)
        hs = slice(gq * 512, (gq + 1) * 512)
        Wq = (w_attn[:, 0:2048][:, hs] * SCALE).astype(np.float32)
        Wk = w_attn[:, 2048:4096][:, hs]
        Wv = w_attn[:, 4096:6144][:, hs]
        WQK = np.concatenate([Wq, Wk], axis=1)            # [2048, 1024]
        wqk = np.ascontiguousarray(
            WQK.reshape(16, 128, 8, 128).transpose(1, 2, 0, 3)).astype(
            np.float16)                                   # ki mi ko mc
        wv = np.ascontiguousarray(
            Wv.reshape(16, 128, 512).transpose(1, 0, 2)).astype(np.float16)
        wPr = np.ascontiguousarray(
            w_proj[hs, :].reshape(4, 128, 2048).transpose(1, 0, 2)).astype(
            np.float16)
        xTr = np.ascontiguousarray(
            x[b].T.reshape(16, 128, 1024).transpose(1, 0, 2)).astype(
            np.float16)
        in_maps.append({
            "xT": xTr, "wQK": wqk, "wV": wv, "wP": wPr,
            "cosT": cosT, "sinT": sinT, "tri4": tri4,
        })
    return in_maps


def kernel(x, pos, w_attn, w_proj, _trace=False):
    global _NC_CACHE
    _install_patches()
    from concourse.bass_utils import run_bass_kernel_spmd

    if _NC_CACHE is None:
        _NC_CACHE = _build_bass()
    nc = _NC_CACHE
    in_maps = _host_inputs(x, pos, w_attn, w_proj)
    res = run_bass_kernel_spmd(nc, in_maps, core_ids=list(range(8)), trace=_trace)
    outs = [np.asarray(res.results[c]["out"], dtype=np.float32)
            for c in range(8)]
    full = np.stack([
        outs[0] + outs[1] + outs[2] + outs[3],
        outs[4] + outs[5] + outs[6] + outs[7],
    ]).astype(np.float32)
    kernel.last_results = res
    return full


# revision 35
# speedup vs baseline: 1.1911x; 1.1367x over previous
"""Causal self-attention (RoPE quirk variant) on 8 Trainium2 NeuronCores.

Reference computation (B=2, T=1024, C=2048, H=64 heads, hd=32):
  qkv = x @ w_attn; split q,k,v; RoPE(dim=n_head quirk) on q,k;
  causal softmax attention; y @ w_proj.

Sharding: 8 cores = 2 batches x 4 head-quarter shards (16 heads / core).
Each core computes attention for its 16 heads on its batch and a partial
output projection (its 512 channels of the 2048-channel contraction);
the host sums the 4 partials per batch.

Device design (v3):
  * fp16 everywhere on the 16-bit path; PSUM/z stay fp32.
  * v is stored augmented per head as 64-wide stationary tiles:
    head-even [v(0:32) | ones(32:64)], head-odd [ones(0:32) | v(32:64)].
    A head pair occupies one psum bank as [y0 | z0 | z1 | y1] (pair at
    tile_position cols 0 / 64), so the softmax denominators ride along
    in the PV matmul -- the separate ones-matmul z pass of v2 is gone
    -- while keeping the narrow (128x64) PE tile shape that streams
    ~2x faster than full (128x128) tiles.
  * y normalization without the 3.3us DVE reciprocal: 1/z computed on
    the scalar engine as Exp(-Ln(z)) over the whole psum tile (unused
    rows produce garbage that is never read), then per-head-band
    [32,512] DVE multiplies.
  * attention groups cascade: each segment runs qc1 of group g plus
    qc0 of group g+1 against the next group's projection filler, so
    the ACT-bound exp chains always overlap PE-heavy projections.
  * input DMAs split across sync/scalar/gpsimd queues with the
    group-0 weights split into 4-ko pieces so the first matmuls can
    start as soon as possible.
  * fillers front-loaded (rope emitted before the trailing v half) so
    rope_g completes well before att_g starts.
  * out DMAs on the sync queue (idle at tail); out-proj casts split
    across vector/scalar; seg F borrows the attention psum banks for a
    4-deep out-proj pipeline.
"""

import json
import os
import sys
from itertools import chain

sys.path.insert(0, "/opt/trn_rl_repo")

import numpy as np

import concourse.bass as bass
import concourse.mybir as mybir
import concourse.tile as tile

F32 = mybir.dt.float32
F16 = mybir.dt.float16

INTERLEAVE = True

B, T, C = 2, 1024, 2048
H, HD = 64, 32
SCALE = 1.0 / np.sqrt(32.0)

_PATCHED = False


def _split_excess_waits(bir_json: bytes) -> bytes:
    """The walrus build in this container encodes at most ONE sync-wait per
    instruction; Tile's wait assigner emits several. Hoist excess waits onto
    same-engine NoOps placed immediately before the instruction."""
    d = json.loads(bir_json)
    ctr = 0
    for fn in d.get("functions", []):
        for blk in fn.get("blocks", []):
            out = []
            for inst in blk.get("instructions", []):
                si = inst.get("sync_info")
                waits = (si or {}).get("on_wait") or []
                if len(waits) > 1:
                    for w in waits[:-1]:
                        out.append({
                            "name": f"WSplit-{ctr}",
                            "opcode": "NoOp",
                            "engine": inst["engine"],
                            "ins": [],
                            "outs": [],
                            "sync_info": {"on_update": [], "on_wait": [w]},
                        })
                        ctr += 1
                    si["on_wait"] = [waits[-1]]
                out.append(inst)
            blk["instructions"] = out
    return json.dumps(d).encode()


def _install_patches():
    global _PATCHED
    if _PATCHED:
        return
    import concourse.bass_utils as bu
    import concourse.bass2jax as b2j

    orig = bu.compile_bir_kernel

    def patched_compile(bir_json, tmpdir, neff_name="file.neff"):
        return orig(_split_excess_waits(bir_json), tmpdir, neff_name)

    bu.compile_bir_kernel = patched_compile
    b2j.compile_bir_kernel = patched_compile
    _PATCHED = True


def _build_bass():
    nc = bass.Bass(trn_type="TRN2")
    xT = nc.dram_tensor("xT", [128, 16, 1024], F16, kind="ExternalInput").ap()
    wQK = nc.dram_tensor("wQK", [128, 8, 16, 128], F16, kind="ExternalInput").ap()
    wV = nc.dram_tensor("wV", [128, 16, 512], F16, kind="ExternalInput").ap()
    wP = nc.dram_tensor("wP", [128, 4, 2048], F16, kind="ExternalInput").ap()
    cosT = nc.dram_tensor("cosT", [128, 1024], F16, kind="ExternalInput").ap()
    sinT = nc.dram_tensor("sinT", [128, 1024], F16, kind="ExternalInput").ap()
    tri4 = nc.dram_tensor("tri4", [128, 4, 128], F16, kind="ExternalInput").ap()
    out = nc.dram_tensor("out", [1024, 2048], F16, kind="ExternalOutput").ap()
    outr = out.rearrange("(tq p) n -> tq p n", p=128)
    debug = os.environ.get("KBG_DEBUG") == "1"
    if debug:
        dbgq = nc.dram_tensor("dbgq", [128, 8, 1024], F16,
                              kind="ExternalOutput").ap()
        dbgv = nc.dram_tensor("dbgv", [128, 8, 1024], F16,
                              kind="ExternalOutput").ap()
        dbgy = nc.dram_tensor("dbgy", [128, 4, 1024], F16,
                              kind="ExternalOutput").ap()
        dbge = nc.dram_tensor("dbge", [128, 4, 512], F16,
                              kind="ExternalOutput").ap()

    EXP = mybir.ActivationFunctionType.Exp

    with tile.TileContext(nc) as tc:
        with tc.tile_pool(name="persist", bufs=1) as persist, \
             tc.tile_pool(name="xpool", bufs=1) as xpool, \
             tc.tile_pool(name="wstream", bufs=3) as wsp, \
             tc.tile_pool(name="qtp", bufs=2) as qtp, \
             tc.tile_pool(name="esp", bufs=4) as esp, \
             tc.tile_pool(name="osb", bufs=4) as osb, \
             tc.tile_pool(name="zrp", bufs=2) as zrp, \
             tc.tile_pool(name="psA", bufs=1, space="PSUM") as psa, \
             tc.tile_pool(name="psS", bufs=2, space="PSUM") as psS, \
             tc.tile_pool(name="psY", bufs=1, space="PSUM") as psY:

            qkT = persist.tile([128, 8, 1024], F16)     # rotated q (0-3) / k (4-7)
            # v augmented: per (kb, head) a 64-wide stationary tile
            # (see module docstring).
            v_sb = persist.tile([128, 8, 16, 64], F16)
            cos_sb = persist.tile([128, 1024], F16)
            sin_sb = persist.tile([128, 1024], F16)     # sign-folded
            tri_sb = persist.tile([128, 4, 128], F16)
            wv_sb = persist.tile([128, 16, 512], F16)
            wp_sb = persist.tile([128, 4, 2048], F16)
            warm = persist.tile([128, 16], F32)
            y_tiles = [persist.tile([128, 1024], F16, name=f"y{g}")
                       for g in range(4)]
            xt = xpool.tile([128, 16, 1024], F16)

            # --- bulk input DMAs first (descriptor pipelines take ~6-9us
            # to produce data; issue the critical path ASAP).
            # sync queue: group-0 q weights (4 small pieces), then even x
            # slices; scalar queue: odd x slices, group-0 k weights, the
            # rope/mask tables, then v weights.  Splitting x across both
            # queues halves the issue-serialization on the critical path.
            wt_first = [wsp.tile([128, 16, 128], F16, tag="wa", name=f"wt0_{j}")
                        for j in range(2)]
            for p in range(4):
                nc.sync.dma_start(wt_first[0][:, 4 * p:4 * p + 4, :],
                                  wQK[:, 0, 4 * p:4 * p + 4, :])
            for ko in range(0, 16, 2):
                nc.sync.dma_start(xt[:, ko, :], xT[:, ko, :])
            for ko in range(1, 16, 2):
                nc.scalar.dma_start(xt[:, ko, :], xT[:, ko, :])
            for p in range(4):
                nc.scalar.dma_start(wt_first[1][:, 4 * p:4 * p + 4, :],
                                    wQK[:, 4, 4 * p:4 * p + 4, :])
            nc.scalar.dma_start(cos_sb, cosT)
            nc.scalar.dma_start(sin_sb, sinT)
            nc.scalar.dma_start(tri_sb, tri4)
            for p in range(2):
                nc.scalar.dma_start(wv_sb[:, 8 * p:8 * p + 8, :],
                                    wV[:, 8 * p:8 * p + 8, :])

            # --- prelude compute: ones bands of v_aug + ACT exp warm-up.
            # Each head's 64-wide stationary tile is [v | 1] (even) /
            # [1 | v] (odd); the v copies fill the v bands later.
            ones_view = v_sb.rearrange("p a (pr tw) c -> p (a pr) tw c", tw=2)
            nc.vector.memset(ones_view[:, :, 0, 32:64], 1.0)
            nc.vector.memset(ones_view[:, :, 1, 0:32], 1.0)
            nc.vector.memset(warm, 0.0)
            nc.scalar.activation(warm, warm, EXP)

            # ---------- generators (each yield ~ one PE-instruction step) ----

            def gen_qk_dma(g):
                if g == 0:
                    yield
                    return
                for j, mi in enumerate((g, 4 + g)):
                    wt = wsp.tile([128, 16, 128], F16, tag="wa",
                                  name=f"wt{g}_{j}")
                    nc.sync.dma_start(wt, wQK[:, mi])
                    gen_qk_dma.cache[(g, j)] = wt
                    yield
            gen_qk_dma.cache = {}

            def gen_qk_mms(g):
                """q/k projection for group g into pre, then RoPE -> qkT."""
                pre = qtp.tile([128, 2, 1024], F16, tag="pre")
                swp = qtp.tile([128, 2, 1024], F16, tag="swp")
                for j in range(2):
                    if g == 0:
                        wt = wt_first[j]
                    else:
                        wt = gen_qk_dma.cache[(g, j)]
                    for half in range(2):
                        ps = psa.tile([128, 512], F32, tag=f"psA{half}",
                                      name=f"qk{g}_{j}_{half}")
                        c0 = half * 512
                        for ko in range(16):
                            nc.tensor.matmul(ps, wt[:, ko, :],
                                             xt[:, ko, c0:c0 + 512],
                                             start=ko == 0, stop=ko == 15)
                            yield
                        nc.vector.tensor_copy(pre[:, j, c0:c0 + 512], ps)
                        yield
                gen_qk_mms.pre[g] = (pre, swp)
            gen_qk_mms.pre = {}

            def gen_rope(g):
                pre, swp = gen_qk_mms.pre[g]
                pre_v = pre.rearrange("(a b) j f -> a b j f", b=2)
                swp_v = swp.rearrange("(a b) j f -> a b j f", b=2)
                nc.sync.dma_start(swp_v[:, 0], pre_v[:, 1])
                nc.sync.dma_start(swp_v[:, 1], pre_v[:, 0])
                yield
                for j, dst in enumerate((qkT[:, g, :], qkT[:, 4 + g, :])):
                    nc.gpsimd.tensor_mul(swp[:, j, :], swp[:, j, :], sin_sb)
                    yield
                    nc.vector.tensor_mul(pre[:, j, :], pre[:, j, :], cos_sb)
                    yield
                    nc.vector.tensor_add(dst, pre[:, j, :], swp[:, j, :])
                    yield

            def gen_v(tb_lo, tb_hi):
                """x-stationary projection: v in natural [T, chan] layout,
                scattered into the augmented v_sb (head stride 64)."""
                for tb in range(tb_lo, tb_hi):
                    psv = psa.tile([128, 512], F32, tag=f"psA{tb % 2}",
                                   name=f"v{tb}")
                    for ko in range(16):
                        nc.tensor.matmul(psv,
                                         xt[:, ko, tb * 128:(tb + 1) * 128],
                                         wv_sb[:, ko, :],
                                         start=ko == 0, stop=ko == 15)
                        yield
                    # scatter: head-even v -> cols 0:32, head-odd -> 32:64
                    src = psv.rearrange("p (pr tw c) -> p pr tw c", tw=2, c=32)
                    dst = v_sb.rearrange(
                        "p a (pr tw) c -> p a pr tw c", tw=2)[:, tb]
                    if tb < 4:
                        nc.scalar.copy(dst[:, :, 0, 0:32], src[:, :, 0])
                        nc.scalar.copy(dst[:, :, 1, 32:64], src[:, :, 1])
                    else:
                        nc.vector.tensor_copy(dst[:, :, 0, 0:32], src[:, :, 0])
                        nc.vector.tensor_copy(dst[:, :, 1, 32:64], src[:, :, 1])
                    yield

            def gen_wp_dma():
                nc.scalar.dma_start(wp_sb, wP)
                yield

            def emit_pv(g, kb, es, N, off, psyA, psyB, start, stop):
                # head pair (2h, 2h+1) in one bank: even at cols 0 (rows
                # [y0|z0]), odd at cols 64 (rows [z1|y1]).
                for h in range(4):
                    psy = psyA if h < 2 else psyB
                    pos = 64 * (h % 2)
                    nc.tensor.matmul(psy[pos:pos + 64, off:512],
                                     v_sb[:, kb, 4 * g + h, :],
                                     es[:, h, :N],
                                     start=start, stop=stop,
                                     tile_position=(0, pos),
                                     skip_group_check=True)

            def gen_att(g, qcs):
                """Attention blocks for group g.  Scores/exp run at head-pair
                granularity: each pair owns a full psum bank per head, and the
                2-bank pair tiles are double-buffered so the next block's
                score matmuls overlap the current block's ACT exp."""
                y_g = y_tiles[g]
                for qc in qcs:
                    q0 = qc * 512
                    nkb = (qc + 1) * 4
                    psyA = psY.tile([128, 512], F32, tag="psyA",
                                    name=f"psyA{g}_{qc}")
                    psyB = psY.tile([128, 512], F32, tag="psyB",
                                    name=f"psyB{g}_{qc}")
                    prev = None
                    for kb in range(nkb):
                        k0 = kb * 128
                        n0 = max(q0, k0)
                        N = q0 + 512 - n0
                        off = n0 - q0
                        es = esp.tile([128, 4, 512], F16, tag="es")
                        pss2 = []
                        for hp in range(2):
                            pss = psS.tile([128, 2, 512], F32, tag="pss")
                            pss2.append(pss)
                            for j in range(2):
                                h = 2 * hp + j
                                nc.tensor.matmul(
                                    pss[:, j, :N],
                                    qkT[32 * h:32 * h + 32, 4 + g,
                                        k0:k0 + 128],
                                    qkT[32 * h:32 * h + 32, g, n0:n0 + N],
                                    start=True, stop=True,
                                    tile_position=(32 * h, 0))
                        for hp in range(2):
                            nc.scalar.activation(
                                es[:, 2 * hp:2 * hp + 2, :N],
                                pss2[hp][:, :, :N], EXP)
                        if prev is not None:
                            emit_pv(*prev)
                        if k0 >= q0:
                            nc.vector.tensor_mul(es[:, :, 0:128],
                                                 es[:, :, 0:128], tri_sb)
                        if debug and g == 0 and qc == 0 and kb == 0:
                            nc.sync.dma_start(dbge, es)
                        prev = (g, kb, es, N, off, psyA, psyB, kb == 0,
                                kb == nkb - 1)
                        yield
                    emit_pv(*prev)
                    # y normalization straight from psum: 1/z = Exp(-Ln z)
                    # on the scalar engine over the whole tile (y rows give
                    # garbage that is never read), then per-head band muls.
                    for pi, psy in enumerate((psyA, psyB)):
                        lnz = zrp.tile([128, 512], F32, tag=f"ln{pi}",
                                       bufs=1, name=f"ln{pi}_{g}_{qc}")
                        zr = zrp.tile([128, 512], F32, tag=f"zr{pi}",
                                      name=f"zr{pi}_{g}_{qc}")
                        nc.scalar.activation(lnz, psy,
                                             mybir.ActivationFunctionType.Ln)
                        nc.scalar.activation(zr, lnz, EXP, scale=-1.0)
                        nc.vector.tensor_mul(
                            y_g[64 * pi:64 * pi + 32, q0:q0 + 512],
                            psy[0:32, :], zr[32:64, :])
                        nc.vector.tensor_mul(
                            y_g[64 * pi + 32:64 * pi + 64, q0:q0 + 512],
                            psy[96:128, :], zr[64:96, :])
                    yield

            def gen_outproj(tq_list, seg_f=False):
                pso_tags = [(psa, "psA0"), (psa, "psA1")]
                if seg_f:
                    pso_tags += [(psY, "psyA"), (psY, "psyB")]
                ctr = 0
                for tq in tq_list:
                    for p in range(2):
                        pso = []
                        for n in range(2):
                            pool, tg = pso_tags[ctr % len(pso_tags)]
                            ctr += 1
                            pso.append(pool.tile([128, 512], F32, tag=tg,
                                                 name=f"pso{tq}_{p}_{n}"))
                        for gk in range(4):
                            lhs = y_tiles[gk][:, tq * 128:(tq + 1) * 128]
                            for n in range(2):
                                nc.tensor.matmul(
                                    pso[n], lhs,
                                    wp_sb[:, gk, (2 * p + n) * 512:
                                          (2 * p + n + 1) * 512],
                                    start=gk == 0, stop=gk == 3)
                                yield
                        for n in range(2):
                            o_sb = osb.tile([128, 512], F16, tag="osb")
                            if n == 1:
                                nc.scalar.copy(o_sb, pso[n])
                            else:
                                nc.vector.tensor_copy(o_sb, pso[n])
                            nc.scalar.dma_start(
                                outr[tq][:, (2 * p + n) * 512:
                                         (2 * p + n + 1) * 512], o_sb)
                            yield

            # ---------- driver: interleave attention with filler PE work ----

            def run(gen):
                for _ in gen:
                    pass

            def co_run(att_gen, filler, per_block):
                """One attention block, then `per_block` filler steps."""
                if not INTERLEAVE:
                    run(filler)
                    run(att_gen)
                    return
                for _ in att_gen:
                    for _ in range(per_block):
                        if next(filler, StopIteration) is StopIteration:
                            break
                run(filler)

            # seg A: q/k proj of group 0 + RoPE0 + first half of v
            run(gen_qk_mms(0))
            run(gen_rope(0))
            run(gen_v(0, 4))

            # Interleaved segments.  EMISSION-ORDER INVARIANT: a consumer
            # must be emitted after its producer (Tile only creates deps in
            # program order) -- so v(4,8) is emitted before att0.qc1's PV
            # blocks, and each rope_g before att_g's first scores.  From
            # seg C on, segments cascade: qc1 of group g plus qc0 of group
            # g+1 run against the next group's projection filler, so the
            # ACT-bound exp chains always overlap PE-heavy projections.
            filler_b = chain(gen_qk_dma(1), gen_v(4, 6), gen_qk_mms(1),
                             gen_rope(1), gen_v(6, 8))
            co_run(gen_att(0, (0, 1)), filler_b, 12)

            filler_c = chain(gen_qk_dma(2), gen_qk_mms(2), gen_rope(2),
                             gen_wp_dma())
            co_run(chain(gen_att(1, (0, 1)), gen_att(2, (0,))), filler_c, 7)

            filler_d = chain(gen_qk_dma(3), gen_qk_mms(3), gen_rope(3))
            co_run(chain(gen_att(2, (1,)), gen_att(3, (0,))), filler_d, 11)

            # seg E: att3 qc1 interleaved with the first half of the
            # out-projection (tq 0-3 only need y3 columns from qc0).
            co_run(gen_att(3, (1,)), gen_outproj((0, 1, 2, 3)), 9)

            # seg F: remaining out-projection (borrows attention psum banks
            # for a 4-deep pipeline; casts on scalar -- ACT is idle now).
            run(gen_outproj((4, 5, 6, 7), seg_f=True))

            if debug:
                nc.sync.dma_start(dbgq, qkT)
                nc.sync.dma_start(dbgv,
                                  v_sb.rearrange("p a h c -> p a (h c)"))
                for g in range(4):
                    nc.sync.dma_start(dbgy[:, g], y_tiles[g])
    return nc


_NC_CACHE = None


def _host_inputs(x, pos, w_attn, w_proj):
    """Build the 8 per-core input dicts."""
    x = np.asarray(x, dtype=np.float32)
    pos = np.asarray(pos, dtype=np.float32)
    w_attn = np.asarray(w_attn, dtype=np.float32)
    w_proj = np.asarray(w_proj, dtype=np.float32)

    TRI = (np.arange(128)[:, None] <= np.arange(128)[None, :]).astype(
        np.float16)
    tri4 = np.ascontiguousarray(np.tile(TRI[:, None, :], (1, 4, 1)))
    inv_freq = (1.0 / (10000.0 ** (np.arange(0, H, 2, dtype=np.float32) / H)))
    sinus = pos[:, None] * inv_freq[None, :]              # [T, 32]
    cosT = np.tile(np.cos(sinus).T, (4, 1))               # [128, T]
    sinT = np.tile(np.sin(sinus).T, (4, 1)).copy()
    sinT[0::2, :] *= -1.0                                 # rotate_half signs
    cosT = cosT.astype(np.float16)
    sinT = sinT.astype(np.float16)

    in_maps = []
    for core in range(8):
        b, gq = divmod(core, 4)
        hs = slice(gq * 512, (gq + 1) * 512)
        Wq = (w_attn[:, 0:2048][:, hs] * SCALE).astype(np.float32)
        Wk = w_attn[:, 2048:4096][:, hs]
        Wv = w_attn[:, 4096:6144][:, hs]
        WQK = np.concatenate([Wq, Wk], axis=1)            # [2048, 1024]
        wqk = np.ascontiguousarray(
            WQK.reshape(16, 128, 8, 128).transpose(1, 2, 0, 3)).astype(
            np.float16)                                   # ki mi ko mc
        wv = np.ascontiguousarray(
            Wv.reshape(16, 128, 512).transpose(1, 0, 2)).astype(np.float16)
        wPr = np.ascontiguousarray(
            w_proj[hs, :].reshape(4, 128, 2048).transpose(1, 0, 2)).astype(
            np.float16)
        xTr = np.ascontiguousarray(
            x[b].T.reshape(16, 128, 1024).transpose(1, 0, 2)).astype(
            np.float16)
        in_maps.append({
            "xT": xTr, "wQK": wqk, "wV": wv, "wP": wPr,
            "cosT": cosT, "sinT": sinT, "tri4": tri4,
        })
    return in_maps


def kernel(x, pos, w_attn, w_proj, _trace=False):
    global _NC_CACHE
    _install_patches()
    from concourse.bass_utils import run_bass_kernel_spmd

    if _NC_CACHE is None:
        _NC_CACHE = _build_bass()
    nc = _NC_CACHE
    in_maps = _host_inputs(x, pos, w_attn, w_proj)
    res = run_bass_kernel_spmd(nc, in_maps, core_ids=list(range(8)), trace=_trace)
    outs = [np.asarray(res.results[c]["out"], dtype=np.float32)
            for c in range(8)]
    full = np.stack([
        outs[0] + outs[1] + outs[2] + outs[3],
        outs[4] + outs[5] + outs[6] + outs[7],
    ]).astype(np.float32)
    kernel.last_results = res
    return full


# revision 36
# speedup vs baseline: 1.2412x; 1.0420x over previous
"""Causal self-attention (RoPE quirk variant) on 8 Trainium2 NeuronCores.

Reference computation (B=2, T=1024, C=2048, H=64 heads, hd=32):
  qkv = x @ w_attn; split q,k,v; RoPE(dim=n_head quirk) on q,k;
  causal softmax attention; y @ w_proj.

Sharding: 8 cores = 2 batches x 4 head-quarter shards (16 heads / core).
Each core computes attention for its 16 heads on its batch and a partial
output projection (its 512 channels of the 2048-channel contraction);
the host sums the 4 partials per batch.

Device design (v3):
  * fp16 everywhere on the 16-bit path; PSUM/z stay fp32.
  * v is stored augmented per head as 64-wide stationary tiles:
    head-even [v(0:32) | ones(32:64)], head-odd [ones(0:32) | v(32:64)].
    A head pair occupies one psum bank as [y0 | z0 | z1 | y1] (pair at
    tile_position cols 0 / 64), so the softmax denominators ride along
    in the PV matmul -- the separate ones-matmul z pass of v2 is gone
    -- while keeping the narrow (128x64) PE tile shape that streams
    ~2x faster than full (128x128) tiles.
  * y normalization without the 3.3us DVE reciprocal: 1/z computed on
    the scalar engine as Exp(-Ln(z)) over the whole psum tile (unused
    rows produce garbage that is never read), then per-head-band
    [32,512] DVE multiplies.
  * attention groups cascade (from seg C): each segment runs qc1 of
    group g plus qc0 of group g+1 against the next group's projection
    filler, so the ACT-bound exp chains always overlap PE-heavy
    projections.
  * input DMAs split across the sync and scalar HWDGE queues with the
    group-0 weights in 4-ko pieces and x interleaved even/odd across
    the queues, so the first matmuls start as soon as possible.
  * EMISSION-ORDER INVARIANT (load-bearing): Tile only creates
    cross-engine dependencies in program order, so every consumer must
    be emitted after its producer.  In particular v(tb) copies are
    emitted before the att blocks whose PV reads them, and rope_g
    before att_g's first scores.  Violating this is a silent
    intermittent data race (reads stale SBUF).
  * out-proj casts alternate vector/scalar; out DMAs on the scalar
    queue (sync was measurably slower); seg F borrows the attention
    psum banks for a 4-deep out-proj pipeline.
"""

import json
import os
import sys
from itertools import chain

sys.path.insert(0, "/opt/trn_rl_repo")

import numpy as np

import concourse.bass as bass
import concourse.mybir as mybir
import concourse.tile as tile

F32 = mybir.dt.float32
F16 = mybir.dt.float16

INTERLEAVE = True

B, T, C = 2, 1024, 2048
H, HD = 64, 32
SCALE = 1.0 / np.sqrt(32.0)

_PATCHED = False


def _split_excess_waits(bir_json: bytes) -> bytes:
    """The walrus build in this container encodes at most ONE sync-wait per
    instruction; Tile's wait assigner emits several. Hoist excess waits onto
    same-engine NoOps placed immediately before the instruction."""
    d = json.loads(bir_json)
    ctr = 0
    for fn in d.get("functions", []):
        for blk in fn.get("blocks", []):
            out = []
            for inst in blk.get("instructions", []):
                si = inst.get("sync_info")
                waits = (si or {}).get("on_wait") or []
                if len(waits) > 1:
                    for w in waits[:-1]:
                        out.append({
                            "name": f"WSplit-{ctr}",
                            "opcode": "NoOp",
                            "engine": inst["engine"],
                            "ins": [],
                            "outs": [],
                            "sync_info": {"on_update": [], "on_wait": [w]},
                        })
                        ctr += 1
                    si["on_wait"] = [waits[-1]]
                out.append(inst)
            blk["instructions"] = out
    return json.dumps(d).encode()


def _install_patches():
    global _PATCHED
    if _PATCHED:
        return
    import concourse.bass_utils as bu
    import concourse.bass2jax as b2j

    orig = bu.compile_bir_kernel

    def patched_compile(bir_json, tmpdir, neff_name="file.neff"):
        return orig(_split_excess_waits(bir_json), tmpdir, neff_name)

    bu.compile_bir_kernel = patched_compile
    b2j.compile_bir_kernel = patched_compile
    _PATCHED = True


def _build_bass():
    nc = bass.Bass(trn_type="TRN2")
    xT = nc.dram_tensor("xT", [128, 16, 1024], F16, kind="ExternalInput").ap()
    wQK = nc.dram_tensor("wQK", [128, 8, 16, 128], F16, kind="ExternalInput").ap()
    wV = nc.dram_tensor("wV", [128, 16, 512], F16, kind="ExternalInput").ap()
    wP = nc.dram_tensor("wP", [128, 4, 2048], F16, kind="ExternalInput").ap()
    cosT = nc.dram_tensor("cosT", [128, 1024], F16, kind="ExternalInput").ap()
    sinT = nc.dram_tensor("sinT", [128, 1024], F16, kind="ExternalInput").ap()
    tri4 = nc.dram_tensor("tri4", [128, 4, 128], F16, kind="ExternalInput").ap()
    out = nc.dram_tensor("out", [1024, 2048], F16, kind="ExternalOutput").ap()
    outr = out.rearrange("(tq p) n -> tq p n", p=128)
    debug = os.environ.get("KBG_DEBUG") == "1"
    if debug:
        dbgq = nc.dram_tensor("dbgq", [128, 8, 1024], F16,
                              kind="ExternalOutput").ap()
        dbgv = nc.dram_tensor("dbgv", [128, 8, 1024], F16,
                              kind="ExternalOutput").ap()
        dbgy = nc.dram_tensor("dbgy", [128, 4, 1024], F16,
                              kind="ExternalOutput").ap()
        dbge = nc.dram_tensor("dbge", [128, 4, 512], F16,
                              kind="ExternalOutput").ap()

    EXP = mybir.ActivationFunctionType.Exp

    with tile.TileContext(nc) as tc:
        with tc.tile_pool(name="persist", bufs=1) as persist, \
             tc.tile_pool(name="xpool", bufs=1) as xpool, \
             tc.tile_pool(name="wstream", bufs=3) as wsp, \
             tc.tile_pool(name="qtp", bufs=2) as qtp, \
             tc.tile_pool(name="esp", bufs=4) as esp, \
             tc.tile_pool(name="osb", bufs=4) as osb, \
             tc.tile_pool(name="zrp", bufs=2) as zrp, \
             tc.tile_pool(name="psA", bufs=1, space="PSUM") as psa, \
             tc.tile_pool(name="psS", bufs=2, space="PSUM") as psS, \
             tc.tile_pool(name="psY", bufs=1, space="PSUM") as psY:

            qkT = persist.tile([128, 8, 1024], F16)     # rotated q (0-3) / k (4-7)
            # v augmented: per (kb, head) a 64-wide stationary tile
            # (see module docstring).
            v_sb = persist.tile([128, 8, 16, 64], F16)
            cos_sb = persist.tile([128, 1024], F16)
            sin_sb = persist.tile([128, 1024], F16)     # sign-folded
            tri_sb = persist.tile([128, 4, 128], F16)
            wv_sb = persist.tile([128, 16, 512], F16)
            wp_sb = persist.tile([128, 4, 2048], F16)
            warm = persist.tile([128, 16], F32)
            y_tiles = [persist.tile([128, 1024], F16, name=f"y{g}")
                       for g in range(4)]
            xt = xpool.tile([128, 16, 1024], F16)

            # --- bulk input DMAs first (descriptor pipelines take ~6-9us
            # to produce data; issue the critical path ASAP).
            # sync queue: group-0 q weights (4 small pieces), then even x
            # slices; scalar queue: odd x slices, group-0 k weights, the
            # rope/mask tables, then v weights.  Splitting x across both
            # queues halves the issue-serialization on the critical path.
            wt_first = [wsp.tile([128, 16, 128], F16, tag="wa", name=f"wt0_{j}")
                        for j in range(2)]
            for p in range(4):
                nc.sync.dma_start(wt_first[0][:, 4 * p:4 * p + 4, :],
                                  wQK[:, 0, 4 * p:4 * p + 4, :])
            for ko in range(0, 16, 2):
                nc.sync.dma_start(xt[:, ko, :], xT[:, ko, :])
            for ko in range(1, 16, 2):
                nc.scalar.dma_start(xt[:, ko, :], xT[:, ko, :])
            for p in range(4):
                nc.scalar.dma_start(wt_first[1][:, 4 * p:4 * p + 4, :],
                                    wQK[:, 4, 4 * p:4 * p + 4, :])
            nc.scalar.dma_start(cos_sb, cosT)
            nc.scalar.dma_start(sin_sb, sinT)
            nc.scalar.dma_start(tri_sb, tri4)
            for p in range(2):
                nc.scalar.dma_start(wv_sb[:, 8 * p:8 * p + 8, :],
                                    wV[:, 8 * p:8 * p + 8, :])

            # --- prelude compute: ones bands of v_aug + ACT exp warm-up.
            # Each head's 64-wide stationary tile is [v | 1] (even) /
            # [1 | v] (odd); the v copies fill the v bands later.
            ones_view = v_sb.rearrange("p a (pr tw) c -> p (a pr) tw c", tw=2)
            nc.vector.memset(ones_view[:, :, 0, 32:64], 1.0)
            nc.vector.memset(ones_view[:, :, 1, 0:32], 1.0)
            nc.vector.memset(warm, 0.0)
            nc.scalar.activation(warm, warm, EXP)

            # ---------- generators (each yield ~ one PE-instruction step) ----

            def gen_qk_dma(g):
                if g == 0:
                    yield
                    return
                for j, mi in enumerate((g, 4 + g)):
                    wt = wsp.tile([128, 16, 128], F16, tag="wa",
                                  name=f"wt{g}_{j}")
                    nc.sync.dma_start(wt, wQK[:, mi])
                    gen_qk_dma.cache[(g, j)] = wt
                    yield
            gen_qk_dma.cache = {}

            def gen_qk_mms(g):
                """q/k projection for group g into pre, then RoPE -> qkT."""
                pre = qtp.tile([128, 2, 1024], F16, tag="pre")
                swp = qtp.tile([128, 2, 1024], F16, tag="swp")
                for j in range(2):
                    if g == 0:
                        wt = wt_first[j]
                    else:
                        wt = gen_qk_dma.cache[(g, j)]
                    for half in range(2):
                        ps = psa.tile([128, 512], F32, tag=f"psA{half}",
                                      name=f"qk{g}_{j}_{half}")
                        c0 = half * 512
                        for ko in range(16):
                            nc.tensor.matmul(ps, wt[:, ko, :],
                                             xt[:, ko, c0:c0 + 512],
                                             start=ko == 0, stop=ko == 15)
                            yield
                        nc.vector.tensor_copy(pre[:, j, c0:c0 + 512], ps)
                        yield
                gen_qk_mms.pre[g] = (pre, swp)
            gen_qk_mms.pre = {}

            def gen_rope(g):
                pre, swp = gen_qk_mms.pre[g]
                pre_v = pre.rearrange("(a b) j f -> a b j f", b=2)
                swp_v = swp.rearrange("(a b) j f -> a b j f", b=2)
                nc.sync.dma_start(swp_v[:, 0], pre_v[:, 1])
                nc.sync.dma_start(swp_v[:, 1], pre_v[:, 0])
                yield
                for j, dst in enumerate((qkT[:, g, :], qkT[:, 4 + g, :])):
                    nc.gpsimd.tensor_mul(swp[:, j, :], swp[:, j, :], sin_sb)
                    yield
                    nc.vector.tensor_mul(pre[:, j, :], pre[:, j, :], cos_sb)
                    yield
                    nc.vector.tensor_add(dst, pre[:, j, :], swp[:, j, :])
                    yield

            def gen_v(tb_lo, tb_hi):
                """x-stationary projection: v in natural [T, chan] layout,
                scattered into the augmented v_sb (head stride 64)."""
                for tb in range(tb_lo, tb_hi):
                    psv = psa.tile([128, 512], F32, tag=f"psA{tb % 2}",
                                   name=f"v{tb}")
                    for ko in range(16):
                        nc.tensor.matmul(psv,
                                         xt[:, ko, tb * 128:(tb + 1) * 128],
                                         wv_sb[:, ko, :],
                                         start=ko == 0, stop=ko == 15)
                        yield
                    # scatter: head-even v -> cols 0:32, head-odd -> 32:64
                    src = psv.rearrange("p (pr tw c) -> p pr tw c", tw=2, c=32)
                    dst = v_sb.rearrange(
                        "p a (pr tw) c -> p a pr tw c", tw=2)[:, tb]
                    if tb < 4:
                        nc.scalar.copy(dst[:, :, 0, 0:32], src[:, :, 0])
                        nc.scalar.copy(dst[:, :, 1, 32:64], src[:, :, 1])
                    else:
                        nc.vector.tensor_copy(dst[:, :, 0, 0:32], src[:, :, 0])
                        nc.vector.tensor_copy(dst[:, :, 1, 32:64], src[:, :, 1])
                    yield

            def gen_wp_dma():
                nc.scalar.dma_start(wp_sb, wP)
                yield

            def emit_pv(g, kb, es, N, off, psyA, psyB, start, stop):
                # head pair (2h, 2h+1) in one bank: even at cols 0 (rows
                # [y0|z0]), odd at cols 64 (rows [z1|y1]).
                for h in range(4):
                    psy = psyA if h < 2 else psyB
                    pos = 64 * (h % 2)
                    nc.tensor.matmul(psy[pos:pos + 64, off:512],
                                     v_sb[:, kb, 4 * g + h, :],
                                     es[:, h, :N],
                                     start=start, stop=stop,
                                     tile_position=(0, pos),
                                     skip_group_check=True)

            def gen_att(g, qcs):
                """Attention blocks for group g.  Scores/exp run at head-pair
                granularity: each pair owns a full psum bank per head, and the
                2-bank pair tiles are double-buffered so the next block's
                score matmuls overlap the current block's ACT exp."""
                y_g = y_tiles[g]
                for qc in qcs:
                    q0 = qc * 512
                    nkb = (qc + 1) * 4
                    psyA = psY.tile([128, 512], F32, tag="psyA",
                                    name=f"psyA{g}_{qc}")
                    psyB = psY.tile([128, 512], F32, tag="psyB",
                                    name=f"psyB{g}_{qc}")
                    prev = None
                    for kb in range(nkb):
                        k0 = kb * 128
                        n0 = max(q0, k0)
                        N = q0 + 512 - n0
                        off = n0 - q0
                        es = esp.tile([128, 4, 512], F16, tag="es")
                        pss2 = []
                        for hp in range(2):
                            pss = psS.tile([128, 2, 512], F32, tag="pss")
                            pss2.append(pss)
                            for j in range(2):
                                h = 2 * hp + j
                                nc.tensor.matmul(
                                    pss[:, j, :N],
                                    qkT[32 * h:32 * h + 32, 4 + g,
                                        k0:k0 + 128],
                                    qkT[32 * h:32 * h + 32, g, n0:n0 + N],
                                    start=True, stop=True,
                                    tile_position=(32 * h, 0))
                        for hp in range(2):
                            nc.scalar.activation(
                                es[:, 2 * hp:2 * hp + 2, :N],
                                pss2[hp][:, :, :N], EXP)
                        if prev is not None:
                            emit_pv(*prev)
                        if k0 >= q0:
                            nc.vector.tensor_mul(es[:, :, 0:128],
                                                 es[:, :, 0:128], tri_sb)
                        if debug and g == 0 and qc == 0 and kb == 0:
                            nc.sync.dma_start(dbge, es)
                        prev = (g, kb, es, N, off, psyA, psyB, kb == 0,
                                kb == nkb - 1)
                        yield
                    emit_pv(*prev)
                    # y normalization straight from psum: 1/z = Exp(-Ln z)
                    # on the scalar engine over the whole tile (y rows give
                    # garbage that is never read), then per-head band muls.
                    for pi, psy in enumerate((psyA, psyB)):
                        lnz = zrp.tile([128, 512], F32, tag=f"ln{pi}",
                                       bufs=1, name=f"ln{pi}_{g}_{qc}")
                        zr = zrp.tile([128, 512], F32, tag=f"zr{pi}",
                                      name=f"zr{pi}_{g}_{qc}")
                        nc.scalar.activation(lnz, psy,
                                             mybir.ActivationFunctionType.Ln)
                        nc.scalar.activation(zr, lnz, EXP, scale=-1.0)
                        nc.vector.tensor_mul(
                            y_g[64 * pi:64 * pi + 32, q0:q0 + 512],
                            psy[0:32, :], zr[32:64, :])
                        nc.vector.tensor_mul(
                            y_g[64 * pi + 32:64 * pi + 64, q0:q0 + 512],
                            psy[96:128, :], zr[64:96, :])
                    yield

            def gen_outproj(tq_list, seg_f=False):
                pso_tags = [(psa, "psA0"), (psa, "psA1")]
                if seg_f:
                    pso_tags += [(psY, "psyA"), (psY, "psyB")]
                ctr = 0
                for tq in tq_list:
                    for p in range(2):
                        pso = []
                        for n in range(2):
                            pool, tg = pso_tags[ctr % len(pso_tags)]
                            ctr += 1
                            pso.append(pool.tile([128, 512], F32, tag=tg,
                                                 name=f"pso{tq}_{p}_{n}"))
                        for gk in range(4):
                            lhs = y_tiles[gk][:, tq * 128:(tq + 1) * 128]
                            for n in range(2):
                                nc.tensor.matmul(
                                    pso[n], lhs,
                                    wp_sb[:, gk, (2 * p + n) * 512:
                                          (2 * p + n + 1) * 512],
                                    start=gk == 0, stop=gk == 3)
                                yield
                        for n in range(2):
                            o_sb = osb.tile([128, 512], F16, tag="osb")
                            if n == 1:
                                nc.scalar.copy(o_sb, pso[n])
                            else:
                                nc.vector.tensor_copy(o_sb, pso[n])
                            nc.scalar.dma_start(
                                outr[tq][:, (2 * p + n) * 512:
                                         (2 * p + n + 1) * 512], o_sb)
                            yield

            # ---------- driver: interleave attention with filler PE work ----

            def run(gen):
                for _ in gen:
                    pass

            def co_run(att_gen, filler, per_block):
                """One attention block, then `per_block` filler steps."""
                if not INTERLEAVE:
                    run(filler)
                    run(att_gen)
                    return
                for _ in att_gen:
                    for _ in range(per_block):
                        if next(filler, StopIteration) is StopIteration:
                            break
                run(filler)

            # seg A: q/k proj of group 0 + RoPE0 + first half of v
            run(gen_qk_mms(0))
            run(gen_rope(0))
            run(gen_v(0, 4))

            # Interleaved segments.  EMISSION-ORDER INVARIANT: a consumer
            # must be emitted after its producer (Tile only creates deps in
            # program order) -- so v(4,8) is emitted before att0.qc1's PV
            # blocks, and each rope_g before att_g's first scores.  From
            # seg C on, segments cascade: qc1 of group g plus qc0 of group
            # g+1 run against the next group's projection filler, so the
            # ACT-bound exp chains always overlap PE-heavy projections.
            filler_b = chain(gen_qk_dma(1), gen_v(4, 6), gen_qk_mms(1),
                             gen_rope(1), gen_v(6, 8))
            co_run(gen_att(0, (0, 1)), filler_b, 12)

            filler_c = chain(gen_qk_dma(2), gen_qk_mms(2), gen_rope(2),
                             gen_wp_dma())
            co_run(chain(gen_att(1, (0, 1)), gen_att(2, (0,))), filler_c, 7)

            filler_d = chain(gen_qk_dma(3), gen_qk_mms(3), gen_rope(3))
            co_run(chain(gen_att(2, (1,)), gen_att(3, (0,))), filler_d, 11)

            # seg E: att3 qc1 interleaved with the first half of the
            # out-projection (tq 0-3 only need y3 columns from qc0).
            co_run(gen_att(3, (1,)), gen_outproj((0, 1, 2, 3)), 9)

            # seg F: remaining out-projection (borrows attention psum banks
            # for a 4-deep pipeline; casts on scalar -- ACT is idle now).
            run(gen_outproj((4, 5, 6, 7), seg_f=True))

            if debug:
                nc.sync.dma_start(dbgq, qkT)
                nc.sync.dma_start(dbgv,
                                  v_sb.rearrange("p a h c -> p a (h c)"))
                for g in range(4):
                    nc.sync.dma_start(dbgy[:, g], y_tiles[g])
    return nc


_NC_CACHE = None


def _host_inputs(x, pos, w_attn, w_proj):
    """Build the 8 per-core input dicts."""
    x = np.asarray(x, dtype=np.float32)
    pos = np.asarray(pos, dtype=np.float32)
    w_attn = np.asarray(w_attn, dtype=np.float32)
    w_proj = np.asarray(w_proj, dtype=np.float32)

    TRI = (np.arange(128)[:, None] <= np.arange(128)[None, :]).astype(
        np.float16)
    tri4 = np.ascontiguousarray(np.tile(TRI[:, None, :], (1, 4, 1)))
    inv_freq = (1.0 / (10000.0 ** (np.arange(0, H, 2, dtype=np.float32) / H)))
    sinus = pos[:, None] * inv_freq[None, :]              # [T, 32]
    cosT = np.tile(np.cos(sinus).T, (4, 1))               # [128, T]
    sinT = np.tile(np.sin(sinus).T, (4, 1)).copy()
    sinT[0::2, :] *= -1.0                                 # rotate_half signs
    cosT = cosT.astype(np.float16)
    sinT = sinT.astype(np.float16)

    in_maps = []
    for core in range(8):
        b, gq = divmod(core, 4)
        hs = slice(gq * 512, (gq + 1) * 512)
        Wq = (w_attn[:, 0:2048][:, hs] * SCALE).astype(np.float32)
        Wk = w_attn[:, 2048:4096][:, hs]
        Wv = w_attn[:, 4096:6144][:, hs]
        WQK = np.concatenate([Wq, Wk], axis=1)            # [2048, 1024]
        wqk = np.ascontiguousarray(
            WQK.reshape(16, 128, 8, 128).transpose(1, 2, 0, 3)).astype(
            np.float16)                                   # ki mi ko mc
        wv = np.ascontiguousarray(
            Wv.reshape(16, 128, 512).transpose(1, 0, 2)).astype(np.float16)
        wPr = np.ascontiguousarray(
            w_proj[hs, :].reshape(4, 128, 2048).transpose(1, 0, 2)).astype(
            np.float16)
        xTr = np.ascontiguousarray(
            x[b].T.reshape(16, 128, 1024).transpose(1, 0, 2)).astype(
            np.float16)
        in_maps.append({
            "xT": xTr, "wQK": wqk, "wV": wv, "wP": wPr,
            "cosT": cosT, "sinT": sinT, "tri4": tri4,
        })
    return in_maps


def kernel(x, pos, w_attn, w_proj, _trace=False):
    global _NC_CACHE
    _install_patches()
    from concourse.bass_utils import run_bass_kernel_spmd

    if _NC_CACHE is None:
        _NC_CACHE = _build_bass()
    nc = _NC_CACHE
    in_maps = _host_inputs(x, pos, w_attn, w_proj)
    res = run_bass_kernel_spmd(nc, in_maps, core_ids=list(range(8)), trace=_trace)
    outs = [np.asarray(res.results[c]["out"], dtype=np.float32)
            for c in range(8)]
    full = np.stack([
        outs[0] + outs[1] + outs[2] + outs[3],
        outs[4] + outs[5] + outs[6] + outs[7],
    ]).astype(np.float32)
    kernel.last_results = res
    return full
